# revision 10
# baseline (speedup 1.0000x reference)
"""AffCoeffToMatrix TRN2 kernel (packed v2).

For each batch element (B = 2,000,000):
  R = rodrigues(rotat), U = rodrigues(scal_dir), D = exp(scal)
  M = R @ (U @ diag(D) @ U^T);  out = [M | trans]  -> [B, 3, 4] f32

Sharding: pure batch-parallel over 8 NeuronCores (no communication).
On-core layout: batch spread over [128 partitions x F free]. The f32 scalar
chain for BOTH rotations runs as paired [2F] ops (ln/exp for sqrt+recip,
ACT Sin with compare+STT range wrap). The matrix phase is fp16 planar in
row-major multi-plane tiles so single DVE instructions (2x mode) process
3-9 planes via affine access patterns; ScalarE (ACT) carries all
transcendentals, deinterleave, squares, and output interleave copies.
"""
import math
import sys

for _p in ("/opt/trn_rl_repo", "/root/.axon_site/_ro/trn_rl_repo"):
    if _p not in sys.path:
        sys.path.append(_p)

import numpy as np

import concourse.bass as bass
import concourse.mybir as mybir
import concourse.tile as tile

F32 = mybir.dt.float32
F16 = mybir.dt.float16
AF = mybir.ActivationFunctionType
OP = mybir.AluOpType
PI = math.pi

# ---- hardcoded problem geometry ----
B = 2_000_000
N_CORES = 8
P = 128
F = 328            # free-dim elements per tile
T = 6              # tiles per core
L = F * T          # elements per partition lane
E = P * L          # elements per core
BPAD = N_CORES * E

MAT_DT = F16


def _split_multi_waits(nc, limit=1, drain_limit=0):
    """This container's walrus cannot encode >1 sync-wait per instruction
    (Drain: none at all). Spill extras onto same-engine NOPs."""
    for b in nc.main_func.blocks:
        new = []
        for ins in b.instructions:
            si = getattr(ins, "sync_info", None)
            waits = list(si.on_wait) if (si is not None and si.on_wait) else []
            lim = drain_limit if isinstance(ins, mybir.InstDrain) else limit
            if len(waits) > lim:
                keep, spill = waits[:lim], waits[lim:]
                for w in spill:
                    nop = mybir.InstNoOp(
                        name=nc.get_next_instruction_name(),
                        sync_info=mybir.SyncInfo(on_wait=[w], on_update=[]),
                        bass_nofuse=True,
                        engine=ins.engine,
                    )
                    nc.register_instruction(nop)
                    new.append(nop)
                ins.sync_info = mybir.SyncInfo(
                    on_wait=keep, on_update=list(si.on_update or [])
                )
            new.append(ins)
        b.instructions[:] = new


def build_module(F=F, T=T, mat_dt=MAT_DT, loop_rep=None):
    nc = bass.Bass()
    E_ = P * F * T
    rot = nc.dram_tensor("rotat", [E_, 3], F32, kind="ExternalInput")
    sd = nc.dram_tensor("scal_dir", [E_, 3], F32, kind="ExternalInput")
    sc = nc.dram_tensor("scal", [E_, 3], F32, kind="ExternalInput")
    tr = nc.dram_tensor("trans", [E_, 3], F32, kind="ExternalInput")
    out = nc.dram_tensor("out", [E_, 12], F32, kind="ExternalOutput")

    rotv = rot[:].rearrange("(t p f) c -> t p (f c)", t=T, p=P)
    sdv = sd[:].rearrange("(t p f) c -> t p (f c)", t=T, p=P)
    scv = sc[:].rearrange("(t p f) c -> t p (f c)", t=T, p=P)
    trv = tr[:].rearrange("(t p f) c -> t p (f c)", t=T, p=P)
    outv = out[:].rearrange("(t p f) c -> t p (f c)", t=T, p=P)

    def bcast(ap_pf, n):
        # [P, F'] -> [P, n, F'] with step-0 middle dim
        return ap_pf.unsqueeze(1).to_broadcast((P, n, ap_pf.shape[-1]))

    with tile.TileContext(nc) as tc:
        with (
            tc.tile_pool(name="pin", bufs=2) as pin,
            tc.tile_pool(name="pout", bufs=2) as pout,
            tc.tile_pool(name="pch", bufs=1) as pch,
            tc.tile_pool(name="pch2", bufs=2) as pch2,
            tc.tile_pool(name="pm2", bufs=2) as pm2,
            tc.tile_pool(name="ppsum", bufs=1, space="PSUM") as ppsum,
            tc.tile_pool(name="pmat", bufs=1) as pmat,
            tc.tile_pool(name="pc", bufs=1) as pc,
        ):
            pi2 = pc.tile([P, 1], F32, tag="pi2")
            nc.vector.memset(pi2[:], PI / 2)

            def part_pre(ti):
                st = {}
                rs6 = pin.tile([P, 6 * F], F32, tag="rs6", name="rs6")
                nc.sync.dma_start(out=rs6[:, : 3 * F], in_=rotv[ti])
                nc.sync.dma_start(out=rs6[:, 3 * F :], in_=sdv[ti])
                c3 = pin.tile([P, 3 * F], F32, tag="sc3", name="sc3")
                nc.sync.dma_start(out=c3[:], in_=scv[ti])
                t3 = pin.tile([P, 3 * F], F32, tag="tr3", name="tr3")
                nc.sync.dma_start(out=t3[:], in_=trv[ti])
                st["rs6"], st["c3"], st["t3"] = rs6, c3, t3

                F2 = 2 * F

                def cht(tag, w=F2):
                    return pch.tile([P, w], F32, tag=tag, name=tag)

                # ---------- paired scalar chain (R | U), NLE table set ----
                sq6 = cht("sq6", 6 * F)
                nc.scalar.activation(sq6[:], rs6[:], AF.Square)
                sqv = sq6[:].rearrange("p (g f c) -> p g c f", g=2, c=3)
                th2a = ppsum.tile([P, F2], F32, tag="th2a", name="th2a")
                th2av = th2a[:].rearrange("p (g f) -> p g f", g=2)
                nc.vector.tensor_add(th2av, sqv[:, :, 0, :], sqv[:, :, 1, :])
                th2 = ppsum.tile([P, F2], F32, tag="th2", name="th2")
                th2v = th2[:].rearrange("p (g f) -> p g f", g=2)
                nc.vector.tensor_add(th2v, th2av, sqv[:, :, 2, :])
                lg = cht("lg")
                nc.scalar.activation(lg[:], th2[:], AF.Ln)
                th = pch2.tile([P, F2], F32, tag="th", name="th")
                nc.scalar.activation(th[:], lg[:], AF.Exp, scale=0.5)
                rth = pch2.tile([P, F2], F32, tag="rth", name="rth")
                nc.scalar.activation(rth[:], lg[:], AF.Exp, scale=-0.5)
                e3 = pm2.tile([P, 3 * F], mat_dt, tag="e3", name="e3")
                e3v = e3[:].rearrange("p (c f) -> p c f", c=3)
                nc.scalar.activation(
                    e3v, c3[:].rearrange("p (f c) -> p c f", c=3), AF.Exp, scale=0.5
                )
                st["th"], st["rth"], st["e3"] = th, rth, e3
                return st

            def part_trig(st):
                F2 = 2 * F
                th = st["th"]
                sh = pch2.tile([P, F2], F32, tag="sh", name="sh")
                nc.scalar.activation(sh[:], th[:], AF.Sin, scale=0.5)
                m = pch.tile([P, F2], F32, tag="m", name="m")
                nc.vector.tensor_scalar(m[:], th[:], PI, None, OP.is_gt)
                u4 = pch.tile([P, F2], F32, tag="u4", name="u4")
                nc.vector.scalar_tensor_tensor(
                    u4[:], m[:], -4 * PI, th[:], OP.mult, OP.add
                )
                ch = pch2.tile([P, F2], F32, tag="ch", name="ch")
                nc.scalar.activation(ch[:], u4[:], AF.Sin, scale=0.5, bias=pi2[:])
                st["sh"], st["ch"] = sh, ch

            def part_mat(ti, st):
                rs6, c3, t3 = st["rs6"], st["c3"], st["t3"]
                sh, ch, rth, e3 = st["sh"], st["ch"], st["rth"], st["e3"]
                e3v = e3[:].rearrange("p (c f) -> p c f", c=3)
                ot = pout.tile([P, 12 * F], F32, tag="out", name="ot")
                otv = ot[:].rearrange("p (f c) -> p c f", c=12)

                F2 = 2 * F

                def cht(tag, w=F2):
                    return pch.tile([P, w], F32, tag=tag, name=tag)

                def mt(tag, w):
                    return pmat.tile([P, w], mat_dt, tag=tag, name=tag)

                t = cht("t")
                nc.vector.tensor_mul(t[:], sh[:], rth[:])
                a2 = mt("a2", F2)
                nc.vector.scalar_tensor_tensor(
                    a2[:], t[:], 2.0, ch[:], OP.mult, OP.mult
                )
                b2 = mt("b2", F2)
                nc.scalar.activation(b2[:], t[:], AF.Square, scale=math.sqrt(2.0))
                sh2d = cht("sh2d")
                nc.scalar.activation(sh2d[:], sh[:], AF.Square, scale=math.sqrt(2.0))
                c2 = mt("c2", F2)
                nc.scalar.activation(c2[:], sh2d[:], AF.Identity, scale=-1.0, bias=1.0)

                # ---------- rotation matrix build (fp16, packed) ----------
                def rotation(g, pref):
                    """g: 0 for R (rotat), 1 for U (scal_dir). Returns the
                    row-major [P, 9F] matrix tile."""
                    vv = rs6[:, 3 * F * g : 3 * F * (g + 1)].rearrange(
                        "p (f c) -> p c f", c=3
                    )
                    av = a2[:, F * g : F * (g + 1)]
                    bv_ = b2[:, F * g : F * (g + 1)]
                    cv = c2[:, F * g : F * (g + 1)]
                    vh5 = mt(pref + "vh5", 5 * F)
                    vh5v = vh5[:].rearrange("p (c f) -> p c f", c=5)
                    # deint: (x|y|z) then (x|y) again for cyclic reads
                    nc.scalar.activation(vh5v[:, 0:3, :], vv, AF.Copy)
                    nc.scalar.activation(vh5v[:, 3:5, :], vv[:, 0:2, :], AF.Copy)
                    bv3 = mt(pref + "bv3", 3 * F)
                    bv3v = bv3[:].rearrange("p (c f) -> p c f", c=3)
                    nc.vector.tensor_mul(bv3v, bcast(bv_, 3), vh5v[:, 0:3, :])
                    # (az, ax, ay) directly: vh5[2:5] = (z, x, y)
                    avc = mt(pref + "avc", 3 * F)
                    avcv = avc[:].rearrange("p (c f) -> p c f", c=3)
                    nc.vector.tensor_mul(avcv, bcast(av, 3), vh5v[:, 2:5, :])
                    d3 = mt(pref + "d3", 3 * F)
                    d3v = d3[:].rearrange("p (c f) -> p c f", c=3)
                    nc.vector.tensor_mul(d3v, bv3v, vh5v[:, 0:3, :])
                    p3 = mt(pref + "p3", 3 * F)
                    p3v = p3[:].rearrange("p (c f) -> p c f", c=3)
                    # (p01, p12, p20) = (bx*y, by*z, bz*x)
                    nc.vector.tensor_mul(p3v, bv3v, vh5v[:, 1:4, :])
                    r9 = mt(pref + "9", 9 * F)
                    r9v = r9[:].rearrange("p (k f) -> p k f", k=9)
                    # diag: R(0,4,8) = d + c
                    nc.vector.tensor_add(r9v[:, 0:9:4, :], d3v, bcast(cv, 3))
                    # plus: R10@3 = p01+az, R21@7 = p12+ax; R02@2 = p20+ay
                    nc.vector.tensor_add(
                        r9v[:, 3:8:4, :], p3v[:, 0:2, :], avcv[:, 0:2, :]
                    )
                    nc.vector.tensor_add(
                        r9v[:, 2, :], p3v[:, 2, :], avcv[:, 2, :]
                    )
                    # minus: R01@1 = p01-az, R12@5 = p12-ax; R20@6 = p20-ay
                    nc.vector.tensor_sub(
                        r9v[:, 1:6:4, :], p3v[:, 0:2, :], avcv[:, 0:2, :]
                    )
                    nc.vector.tensor_sub(
                        r9v[:, 6, :], p3v[:, 2, :], avcv[:, 2, :]
                    )
                    return r9

                R9 = rotation(0, "R")
                U9 = rotation(1, "U")
                R9v = R9[:].rearrange("p (k f) -> p k f", k=9)
                U9v = U9[:].rearrange("p (i k f) -> p i k f", i=3, k=3)

                # ---------- scaling: W = U * diag(e), S = W W^T ----------
                W9 = mt("W9", 9 * F)
                W9v4 = W9[:].rearrange("p (i k f) -> p i k f", i=3, k=3)
                e_b = e3v.unsqueeze(1).to_broadcast((P, 3, 3, F))
                nc.vector.tensor_mul(W9v4, U9v, e_b)
                W9v = W9[:].rearrange("p (k f) -> p k f", k=9)
                sqW = mt("sqW", 9 * F)
                nc.scalar.activation(sqW[:], W9[:], AF.Square)
                sqWv = sqW[:].rearrange("p (i k f) -> p i k f", i=3, k=3)
                S9 = mt("S9", 9 * F)
                S9v = S9[:].rearrange("p (k f) -> p k f", k=9)
                # diagonal
                sdt = mt("sdt", 3 * F)
                sdtv = sdt[:].rearrange("p (c f) -> p c f", c=3)
                nc.vector.tensor_add(sdtv, sqWv[:, :, 0, :], sqWv[:, :, 1, :])
                nc.vector.tensor_add(S9v[:, 0:9:4, :], sdtv, sqWv[:, :, 2, :])
                # off-diagonal products: row pairs (0,1), (0,2), (1,2)
                pp = mt("pp", 9 * F)
                ppv = pp[:].rearrange("p (g k f) -> p g k f", g=3, k=3)
                nc.vector.tensor_mul(
                    ppv[:, 0, :, :], W9v[:, 0:3, :], W9v[:, 3:6, :]
                )
                nc.vector.tensor_mul(
                    ppv[:, 1, :, :], W9v[:, 0:3, :], W9v[:, 6:9, :]
                )
                nc.vector.tensor_mul(
                    ppv[:, 2, :, :], W9v[:, 3:6, :], W9v[:, 6:9, :]
                )
                q3 = mt("q3", 3 * F)
                q3v = q3[:].rearrange("p (g f) -> p g f", g=3)
                nc.vector.tensor_add(q3v, ppv[:, :, 0, :], ppv[:, :, 1, :])
                # S01@1, S02@2 ; S12@5 ; S10@3 ; S20@6, S21@7
                nc.vector.tensor_add(
                    S9v[:, 1:3, :], q3v[:, 0:2, :], ppv[:, 0:2, 2, :]
                )
                nc.vector.tensor_add(S9v[:, 5, :], q3v[:, 2, :], ppv[:, 2, 2, :])
                nc.vector.tensor_add(S9v[:, 3, :], q3v[:, 0, :], ppv[:, 0, 2, :])
                nc.vector.tensor_add(
                    S9v[:, 6:8, :], q3v[:, 1:3, :], ppv[:, 1:3, 2, :]
                )

                # ---------- M = R @ S (per output row, packed 3F) ----------
                m9 = pmat.tile([P, 9 * F], mat_dt, tag="m9", name="m9")
                m9v = m9[:].rearrange("p (k f) -> p k f", k=9)
                for i in range(3):
                    mp1 = mt("mp1", 3 * F)
                    mp1v = mp1[:].rearrange("p (c f) -> p c f", c=3)
                    nc.vector.tensor_mul(
                        mp1v, bcast(R9v[:, 3 * i, :], 3), S9v[:, 0:3, :]
                    )
                    mp2 = mt("mp2", 3 * F)
                    mp2v = mp2[:].rearrange("p (c f) -> p c f", c=3)
                    nc.vector.tensor_mul(
                        mp2v, bcast(R9v[:, 3 * i + 1, :], 3), S9v[:, 3:6, :]
                    )
                    ms = mt("ms", 3 * F)
                    msv = ms[:].rearrange("p (c f) -> p c f", c=3)
                    nc.vector.tensor_add(msv, mp1v, mp2v)
                    mp3 = mt("mp3", 3 * F)
                    mp3v = mp3[:].rearrange("p (c f) -> p c f", c=3)
                    nc.vector.tensor_mul(
                        mp3v, bcast(R9v[:, 3 * i + 2, :], 3), S9v[:, 6:9, :]
                    )
                    nc.vector.tensor_add(m9v[:, 3 * i : 3 * i + 3, :], msv, mp3v)
                # interleave to f32 out: rows + trans
                for i in range(3):
                    nc.scalar.activation(
                        otv[:, 4 * i : 4 * i + 3, :],
                        m9v[:, 3 * i : 3 * i + 3, :],
                        AF.Copy,
                    )
                nc.scalar.activation(
                    otv[:, 3:12:4, :],
                    t3[:].rearrange("p (f c) -> p c f", c=3),
                    AF.Copy,
                )
                nc.sync.dma_start(out=outv[ti], in_=ot[:])

            if loop_rep is None:
                assert T % 2 == 0
                for g in range(0, T, 2):
                    stA = part_pre(g)
                    stB = part_pre(g + 1)
                    part_trig(stA)
                    part_trig(stB)
                    part_mat(g, stA)
                    part_mat(g + 1, stB)
            else:
                with tc.For_i(0, loop_rep, 1, staggered_reset=True):
                    stA = part_pre(0)
                    part_trig(stA)
                    part_mat(0, stA)

    _split_multi_waits(nc)
    return nc


# ----------------------------------------------------------------------------
# host-side execution
# ----------------------------------------------------------------------------
_CACHE = {}


def _get_runner():
    if "runner" in _CACHE:
        return _CACHE["runner"]
    import jax
    from jax.sharding import Mesh, PartitionSpec
    from jax.experimental.shard_map import shard_map
    from concourse.bass2jax import (
        _bass_exec_p,
        install_neuronx_cc_hook,
        partition_id_tensor,
    )

    nc = build_module()
    install_neuronx_cc_hook()
    partition_name = nc.partition_id_tensor.name if nc.partition_id_tensor else None
    in_names, out_names, out_avals, zero_outs = [], [], [], []
    for alloc in nc.m.functions[0].allocations:
        if not isinstance(alloc, mybir.MemoryLocationSet):
            continue
        name = alloc.memorylocations[0].name
        if alloc.kind == "ExternalInput":
            if name != partition_name:
                in_names.append(name)
        elif alloc.kind == "ExternalOutput":
            shape = tuple(alloc.tensor_shape)
            dtype = mybir.dt.np(alloc.dtype)
            out_names.append(name)
            out_avals.append(jax.core.ShapedArray(shape, dtype))
            zero_outs.append(np.zeros(shape, dtype))
    n_params = len(in_names)
    all_in_names = in_names + out_names + (
        [partition_name] if partition_name else []
    )

    def _body(*args):
        operands = list(args)
        if partition_name is not None:
            operands.append(partition_id_tensor())
        outs = _bass_exec_p.bind(
            *operands,
            out_avals=tuple(out_avals),
            in_names=tuple(all_in_names),
            out_names=tuple(out_names),
            lowering_input_output_aliases=(),
            sim_require_finite=True,
            sim_require_nnan=True,
            nc=nc,
        )
        return tuple(outs)

    devices = jax.devices()[:N_CORES]
    mesh = Mesh(np.asarray(devices), ("core",))
    n_outs = len(out_names)
    jf = jax.jit(
        shard_map(
            _body,
            mesh=mesh,
            in_specs=(PartitionSpec("core"),) * (n_params + n_outs),
            out_specs=(PartitionSpec("core"),) * n_outs,
            check_rep=False,
        ),
        donate_argnums=tuple(range(n_params, n_params + n_outs)),
        keep_unused=True,
    )
    _CACHE["runner"] = (jf, in_names, out_names, zero_outs)
    return _CACHE["runner"]


def kernel(trans, rotat, scal_dir, scal):
    jf, in_names, out_names, zero_outs = _get_runner()
    inputs = {"trans": trans, "rotat": rotat, "scal_dir": scal_dir, "scal": scal}
    # pad to BPAD with ones (zeros would make |v| = 0 -> inf/NaN chains)
    padded = {}
    for k, v in inputs.items():
        a = np.ones((BPAD, 3), dtype=np.float32)
        a[:B] = v
        padded[k] = a
    args = [padded[n] for n in in_names]
    zeros = [np.zeros((N_CORES * z.shape[0], *z.shape[1:]), z.dtype) for z in zero_outs]
    outs = jf(*args, *zeros)
    full = np.asarray(outs[0])  # [BPAD, 12]
    return full[:B].reshape(B, 3, 4).astype(np.float32, copy=False)


if __name__ == "__main__":
    rng = np.random.default_rng(0)
    ins = {
        "trans": rng.normal(size=(B, 3)).astype(np.float32),
        "rotat": rng.normal(size=(B, 3)).astype(np.float32),
        "scal_dir": rng.normal(size=(B, 3)).astype(np.float32),
        "scal": rng.normal(size=(B, 3)).astype(np.float32),
    }
    out = kernel(**ins)
    print(out.shape, out.dtype)


# revision 11
# speedup vs baseline: 38071.1973x; 38071.1973x over previous
"""AffCoeffToMatrix TRN2 kernel (packed v2).

For each batch element (B = 2,000,000):
  R = rodrigues(rotat), U = rodrigues(scal_dir), D = exp(scal)
  M = R @ (U @ diag(D) @ U^T);  out = [M | trans]  -> [B, 3, 4] f32

Sharding: pure batch-parallel over 8 NeuronCores (no communication).
On-core layout: batch spread over [128 partitions x F free]. The f32 scalar
chain for BOTH rotations runs as paired [2F] ops (ln/exp for sqrt+recip,
ACT Sin with compare+STT range wrap). The matrix phase is fp16 planar in
row-major multi-plane tiles so single DVE instructions (2x mode) process
3-9 planes via affine access patterns; ScalarE (ACT) carries all
transcendentals, deinterleave, squares, and output interleave copies.
"""
import math
import sys

for _p in ("/opt/trn_rl_repo", "/root/.axon_site/_ro/trn_rl_repo"):
    if _p not in sys.path:
        sys.path.append(_p)

import numpy as np

import concourse.bass as bass
import concourse.mybir as mybir
import concourse.tile as tile

F32 = mybir.dt.float32
F16 = mybir.dt.float16
AF = mybir.ActivationFunctionType
OP = mybir.AluOpType
PI = math.pi

# ---- hardcoded problem geometry ----
B = 2_000_000
N_CORES = 8
P = 128
F = 328            # free-dim elements per tile
T = 6              # tiles per core
L = F * T          # elements per partition lane
E = P * L          # elements per core
BPAD = N_CORES * E

MAT_DT = F16


def _split_multi_waits(nc, limit=1, drain_limit=0):
    """This container's walrus cannot encode >1 sync-wait per instruction
    (Drain: none at all). Spill extras onto same-engine NOPs."""
    for b in nc.main_func.blocks:
        new = []
        for ins in b.instructions:
            si = getattr(ins, "sync_info", None)
            waits = list(si.on_wait) if (si is not None and si.on_wait) else []
            lim = drain_limit if isinstance(ins, mybir.InstDrain) else limit
            if len(waits) > lim:
                keep, spill = waits[:lim], waits[lim:]
                for w in spill:
                    nop = mybir.InstNoOp(
                        name=nc.get_next_instruction_name(),
                        sync_info=mybir.SyncInfo(on_wait=[w], on_update=[]),
                        bass_nofuse=True,
                        engine=ins.engine,
                    )
                    nc.register_instruction(nop)
                    new.append(nop)
                ins.sync_info = mybir.SyncInfo(
                    on_wait=keep, on_update=list(si.on_update or [])
                )
            new.append(ins)
        b.instructions[:] = new


def build_module(F=F, T=T, mat_dt=MAT_DT, loop_rep=None):
    nc = bass.Bass()
    E_ = P * F * T
    rot = nc.dram_tensor("rotat", [E_, 3], F32, kind="ExternalInput")
    sd = nc.dram_tensor("scal_dir", [E_, 3], F32, kind="ExternalInput")
    sc = nc.dram_tensor("scal", [E_, 3], F32, kind="ExternalInput")
    tr = nc.dram_tensor("trans", [E_, 3], F32, kind="ExternalInput")
    out = nc.dram_tensor("out", [E_, 12], F32, kind="ExternalOutput")

    rotv = rot[:].rearrange("(t p f) c -> t p (f c)", t=T, p=P)
    sdv = sd[:].rearrange("(t p f) c -> t p (f c)", t=T, p=P)
    scv = sc[:].rearrange("(t p f) c -> t p (f c)", t=T, p=P)
    trv = tr[:].rearrange("(t p f) c -> t p (f c)", t=T, p=P)
    outv = out[:].rearrange("(t p f) c -> t p (f c)", t=T, p=P)

    def bcast(ap_pf, n):
        # [P, F'] -> [P, n, F'] with step-0 middle dim
        return ap_pf.unsqueeze(1).to_broadcast((P, n, ap_pf.shape[-1]))

    with tile.TileContext(nc) as tc:
        with (
            tc.tile_pool(name="pin", bufs=2) as pin,
            tc.tile_pool(name="pout", bufs=2) as pout,
            tc.tile_pool(name="pch", bufs=1) as pch,
            tc.tile_pool(name="pch2", bufs=2) as pch2,
            tc.tile_pool(name="pm2", bufs=2) as pm2,
            tc.tile_pool(name="ppsum", bufs=1, space="PSUM") as ppsum,
            tc.tile_pool(name="pmat", bufs=1) as pmat,
            tc.tile_pool(name="pc", bufs=1) as pc,
        ):
            pi2 = pc.tile([P, 1], F32, tag="pi2")
            nc.vector.memset(pi2[:], PI / 2)

            def part_pre(ti):
                st = {}
                rs6 = pin.tile([P, 6 * F], F32, tag="rs6", name="rs6")
                nc.sync.dma_start(out=rs6[:, : 3 * F], in_=rotv[ti])
                nc.sync.dma_start(out=rs6[:, 3 * F :], in_=sdv[ti])
                c3 = pin.tile([P, 3 * F], F32, tag="sc3", name="sc3")
                nc.sync.dma_start(out=c3[:], in_=scv[ti])
                t3 = pin.tile([P, 3 * F], F32, tag="tr3", name="tr3")
                nc.sync.dma_start(out=t3[:], in_=trv[ti])
                st["rs6"], st["c3"], st["t3"] = rs6, c3, t3

                F2 = 2 * F

                def cht(tag, w=F2):
                    return pch.tile([P, w], F32, tag=tag, name=tag)

                # ---------- paired scalar chain (R | U), NLE table set ----
                sq6 = cht("sq6", 6 * F)
                nc.scalar.activation(sq6[:], rs6[:], AF.Square)
                sqv = sq6[:].rearrange("p (g f c) -> p g c f", g=2, c=3)
                th2a = ppsum.tile([P, F2], F32, tag="th2a", name="th2a")
                th2av = th2a[:].rearrange("p (g f) -> p g f", g=2)
                nc.vector.tensor_add(th2av, sqv[:, :, 0, :], sqv[:, :, 1, :])
                th2 = ppsum.tile([P, F2], F32, tag="th2", name="th2")
                th2v = th2[:].rearrange("p (g f) -> p g f", g=2)
                nc.vector.tensor_add(th2v, th2av, sqv[:, :, 2, :])
                lg = cht("lg")
                nc.scalar.activation(lg[:], th2[:], AF.Ln)
                th = pch2.tile([P, F2], F32, tag="th", name="th")
                nc.scalar.activation(th[:], lg[:], AF.Exp, scale=0.5)
                rth = pch2.tile([P, F2], F32, tag="rth", name="rth")
                nc.scalar.activation(rth[:], lg[:], AF.Exp, scale=-0.5)
                e3 = pm2.tile([P, 3 * F], mat_dt, tag="e3", name="e3")
                e3v = e3[:].rearrange("p (c f) -> p c f", c=3)
                nc.scalar.activation(
                    e3v, c3[:].rearrange("p (f c) -> p c f", c=3), AF.Exp, scale=0.5
                )
                st["th"], st["rth"], st["e3"] = th, rth, e3
                return st

            def part_trig(st):
                F2 = 2 * F
                th = st["th"]
                sh = pch2.tile([P, F2], F32, tag="sh", name="sh")
                nc.scalar.activation(sh[:], th[:], AF.Sin, scale=0.5)
                m = pch.tile([P, F2], F32, tag="m", name="m")
                nc.vector.tensor_scalar(m[:], th[:], PI, None, OP.is_gt)
                u4 = pch.tile([P, F2], F32, tag="u4", name="u4")
                nc.vector.scalar_tensor_tensor(
                    u4[:], m[:], -4 * PI, th[:], OP.mult, OP.add
                )
                ch = pch2.tile([P, F2], F32, tag="ch", name="ch")
                nc.scalar.activation(ch[:], u4[:], AF.Sin, scale=0.5, bias=pi2[:])
                st["sh"], st["ch"] = sh, ch

            def part_mat(ti, st):
                rs6, c3, t3 = st["rs6"], st["c3"], st["t3"]
                sh, ch, rth, e3 = st["sh"], st["ch"], st["rth"], st["e3"]
                e3v = e3[:].rearrange("p (c f) -> p c f", c=3)
                ot = pout.tile([P, 12 * F], F32, tag="out", name="ot")
                otv = ot[:].rearrange("p (f c) -> p c f", c=12)

                F2 = 2 * F

                def cht(tag, w=F2):
                    return pch.tile([P, w], F32, tag=tag, name=tag)

                def mt(tag, w):
                    return pmat.tile([P, w], mat_dt, tag=tag, name=tag)

                t = cht("t")
                nc.vector.tensor_mul(t[:], sh[:], rth[:])
                a2 = mt("a2", F2)
                nc.vector.scalar_tensor_tensor(
                    a2[:], t[:], 2.0, ch[:], OP.mult, OP.mult
                )
                b2 = mt("b2", F2)
                nc.scalar.activation(b2[:], t[:], AF.Square, scale=math.sqrt(2.0))
                sh2d = cht("sh2d")
                nc.scalar.activation(sh2d[:], sh[:], AF.Square, scale=math.sqrt(2.0))
                c2 = mt("c2", F2)
                nc.scalar.activation(c2[:], sh2d[:], AF.Identity, scale=-1.0, bias=1.0)

                # ---------- rotation matrix build (fp16, packed) ----------
                def rotation(g, pref):
                    """g: 0 for R (rotat), 1 for U (scal_dir). Returns the
                    row-major [P, 9F] matrix tile."""
                    vv = rs6[:, 3 * F * g : 3 * F * (g + 1)].rearrange(
                        "p (f c) -> p c f", c=3
                    )
                    av = a2[:, F * g : F * (g + 1)]
                    bv_ = b2[:, F * g : F * (g + 1)]
                    cv = c2[:, F * g : F * (g + 1)]
                    vh5 = mt(pref + "vh5", 5 * F)
                    vh5v = vh5[:].rearrange("p (c f) -> p c f", c=5)
                    # deint: (x|y|z) then (x|y) again for cyclic reads
                    nc.scalar.activation(vh5v[:, 0:3, :], vv, AF.Copy)
                    nc.scalar.activation(vh5v[:, 3:5, :], vv[:, 0:2, :], AF.Copy)
                    bv3 = mt(pref + "bv3", 3 * F)
                    bv3v = bv3[:].rearrange("p (c f) -> p c f", c=3)
                    nc.vector.tensor_mul(bv3v, bcast(bv_, 3), vh5v[:, 0:3, :])
                    # (az, ax, ay) directly: vh5[2:5] = (z, x, y)
                    avc = mt(pref + "avc", 3 * F)
                    avcv = avc[:].rearrange("p (c f) -> p c f", c=3)
                    nc.vector.tensor_mul(avcv, bcast(av, 3), vh5v[:, 2:5, :])
                    d3 = mt(pref + "d3", 3 * F)
                    d3v = d3[:].rearrange("p (c f) -> p c f", c=3)
                    nc.vector.tensor_mul(d3v, bv3v, vh5v[:, 0:3, :])
                    p3 = mt(pref + "p3", 3 * F)
                    p3v = p3[:].rearrange("p (c f) -> p c f", c=3)
                    # (p01, p12, p20) = (bx*y, by*z, bz*x)
                    nc.vector.tensor_mul(p3v, bv3v, vh5v[:, 1:4, :])
                    r9 = mt(pref + "9", 9 * F)
                    r9v = r9[:].rearrange("p (k f) -> p k f", k=9)
                    # diag: R(0,4,8) = d + c
                    nc.vector.tensor_add(r9v[:, 0:9:4, :], d3v, bcast(cv, 3))
                    # plus: R10@3 = p01+az, R21@7 = p12+ax; R02@2 = p20+ay
                    nc.vector.tensor_add(
                        r9v[:, 3:8:4, :], p3v[:, 0:2, :], avcv[:, 0:2, :]
                    )
                    nc.vector.tensor_add(
                        r9v[:, 2, :], p3v[:, 2, :], avcv[:, 2, :]
                    )
                    # minus: R01@1 = p01-az, R12@5 = p12-ax; R20@6 = p20-ay
                    nc.vector.tensor_sub(
                        r9v[:, 1:6:4, :], p3v[:, 0:2, :], avcv[:, 0:2, :]
                    )
                    nc.vector.tensor_sub(
                        r9v[:, 6, :], p3v[:, 2, :], avcv[:, 2, :]
                    )
                    return r9

                R9 = rotation(0, "R")
                U9 = rotation(1, "U")
                R9v = R9[:].rearrange("p (k f) -> p k f", k=9)
                U9v = U9[:].rearrange("p (i k f) -> p i k f", i=3, k=3)

                # ---------- scaling: W = U * diag(e), S = W W^T ----------
                W9 = mt("W9", 9 * F)
                W9v4 = W9[:].rearrange("p (i k f) -> p i k f", i=3, k=3)
                e_b = e3v.unsqueeze(1).to_broadcast((P, 3, 3, F))
                nc.vector.tensor_mul(W9v4, U9v, e_b)
                W9v = W9[:].rearrange("p (k f) -> p k f", k=9)
                sqW = mt("sqW", 9 * F)
                nc.scalar.activation(sqW[:], W9[:], AF.Square)
                sqWv = sqW[:].rearrange("p (i k f) -> p i k f", i=3, k=3)
                S9 = mt("S9", 9 * F)
                S9v = S9[:].rearrange("p (k f) -> p k f", k=9)
                # diagonal
                sdt = mt("sdt", 3 * F)
                sdtv = sdt[:].rearrange("p (c f) -> p c f", c=3)
                nc.vector.tensor_add(sdtv, sqWv[:, :, 0, :], sqWv[:, :, 1, :])
                nc.vector.tensor_add(S9v[:, 0:9:4, :], sdtv, sqWv[:, :, 2, :])
                # off-diagonal products: row pairs (0,1), (0,2), (1,2)
                pp = mt("pp", 9 * F)
                ppv = pp[:].rearrange("p (g k f) -> p g k f", g=3, k=3)
                nc.vector.tensor_mul(
                    ppv[:, 0, :, :], W9v[:, 0:3, :], W9v[:, 3:6, :]
                )
                nc.vector.tensor_mul(
                    ppv[:, 1, :, :], W9v[:, 0:3, :], W9v[:, 6:9, :]
                )
                nc.vector.tensor_mul(
                    ppv[:, 2, :, :], W9v[:, 3:6, :], W9v[:, 6:9, :]
                )
                q3 = mt("q3", 3 * F)
                q3v = q3[:].rearrange("p (g f) -> p g f", g=3)
                nc.vector.tensor_add(q3v, ppv[:, :, 0, :], ppv[:, :, 1, :])
                # S01@1, S02@2 ; S12@5 ; S10@3 ; S20@6, S21@7
                nc.vector.tensor_add(
                    S9v[:, 1:3, :], q3v[:, 0:2, :], ppv[:, 0:2, 2, :]
                )
                nc.vector.tensor_add(S9v[:, 5, :], q3v[:, 2, :], ppv[:, 2, 2, :])
                nc.vector.tensor_add(S9v[:, 3, :], q3v[:, 0, :], ppv[:, 0, 2, :])
                nc.vector.tensor_add(
                    S9v[:, 6:8, :], q3v[:, 1:3, :], ppv[:, 1:3, 2, :]
                )

                # ---------- M = R @ S (per output row, packed 3F) ----------
                m9 = pmat.tile([P, 9 * F], mat_dt, tag="m9", name="m9")
                m9v = m9[:].rearrange("p (k f) -> p k f", k=9)
                for i in range(3):
                    mp1 = mt("mp1", 3 * F)
                    mp1v = mp1[:].rearrange("p (c f) -> p c f", c=3)
                    nc.vector.tensor_mul(
                        mp1v, bcast(R9v[:, 3 * i, :], 3), S9v[:, 0:3, :]
                    )
                    mp2 = mt("mp2", 3 * F)
                    mp2v = mp2[:].rearrange("p (c f) -> p c f", c=3)
                    nc.vector.tensor_mul(
                        mp2v, bcast(R9v[:, 3 * i + 1, :], 3), S9v[:, 3:6, :]
                    )
                    ms = mt("ms", 3 * F)
                    msv = ms[:].rearrange("p (c f) -> p c f", c=3)
                    nc.vector.tensor_add(msv, mp1v, mp2v)
                    mp3 = mt("mp3", 3 * F)
                    mp3v = mp3[:].rearrange("p (c f) -> p c f", c=3)
                    nc.vector.tensor_mul(
                        mp3v, bcast(R9v[:, 3 * i + 2, :], 3), S9v[:, 6:9, :]
                    )
                    nc.vector.tensor_add(m9v[:, 3 * i : 3 * i + 3, :], msv, mp3v)
                # interleave to f32 out: rows + trans
                for i in range(3):
                    nc.scalar.activation(
                        otv[:, 4 * i : 4 * i + 3, :],
                        m9v[:, 3 * i : 3 * i + 3, :],
                        AF.Copy,
                    )
                nc.scalar.activation(
                    otv[:, 3:12:4, :],
                    t3[:].rearrange("p (f c) -> p c f", c=3),
                    AF.Copy,
                )
                nc.sync.dma_start(out=outv[ti], in_=ot[:])

            if loop_rep is None:
                assert T % 2 == 0
                for g in range(0, T, 2):
                    stA = part_pre(g)
                    stB = part_pre(g + 1)
                    part_trig(stA)
                    part_trig(stB)
                    part_mat(g, stA)
                    part_mat(g + 1, stB)
            else:
                with tc.For_i(0, loop_rep, 1, staggered_reset=True):
                    for g in range(0, T, 2):
                        stA = part_pre(g)
                        stB = part_pre(g + 1)
                        part_trig(stA)
                        part_trig(stB)
                        part_mat(g, stA)
                        part_mat(g + 1, stB)

    _split_multi_waits(nc)
    return nc


# ----------------------------------------------------------------------------
# host-side execution
# ----------------------------------------------------------------------------
_CACHE = {}


def _get_runner():
    if "runner" in _CACHE:
        return _CACHE["runner"]
    import jax
    from jax.sharding import Mesh, PartitionSpec
    from jax.experimental.shard_map import shard_map
    from concourse.bass2jax import (
        _bass_exec_p,
        install_neuronx_cc_hook,
        partition_id_tensor,
    )

    nc = build_module()
    install_neuronx_cc_hook()
    partition_name = nc.partition_id_tensor.name if nc.partition_id_tensor else None
    in_names, out_names, out_avals, zero_outs = [], [], [], []
    for alloc in nc.m.functions[0].allocations:
        if not isinstance(alloc, mybir.MemoryLocationSet):
            continue
        name = alloc.memorylocations[0].name
        if alloc.kind == "ExternalInput":
            if name != partition_name:
                in_names.append(name)
        elif alloc.kind == "ExternalOutput":
            shape = tuple(alloc.tensor_shape)
            dtype = mybir.dt.np(alloc.dtype)
            out_names.append(name)
            out_avals.append(jax.core.ShapedArray(shape, dtype))
            zero_outs.append(np.zeros(shape, dtype))
    n_params = len(in_names)
    all_in_names = in_names + out_names + (
        [partition_name] if partition_name else []
    )

    def _body(*args):
        operands = list(args)
        if partition_name is not None:
            operands.append(partition_id_tensor())
        outs = _bass_exec_p.bind(
            *operands,
            out_avals=tuple(out_avals),
            in_names=tuple(all_in_names),
            out_names=tuple(out_names),
            lowering_input_output_aliases=(),
            sim_require_finite=True,
            sim_require_nnan=True,
            nc=nc,
        )
        return tuple(outs)

    devices = jax.devices()[:N_CORES]
    mesh = Mesh(np.asarray(devices), ("core",))
    n_outs = len(out_names)
    jf = jax.jit(
        shard_map(
            _body,
            mesh=mesh,
            in_specs=(PartitionSpec("core"),) * (n_params + n_outs),
            out_specs=(PartitionSpec("core"),) * n_outs,
            check_rep=False,
        ),
        donate_argnums=tuple(range(n_params, n_params + n_outs)),
        keep_unused=True,
    )
    _CACHE["runner"] = (jf, in_names, out_names, zero_outs)
    return _CACHE["runner"]


def kernel(trans, rotat, scal_dir, scal):
    jf, in_names, out_names, zero_outs = _get_runner()
    inputs = {"trans": trans, "rotat": rotat, "scal_dir": scal_dir, "scal": scal}
    # pad to BPAD with ones (zeros would make |v| = 0 -> inf/NaN chains)
    padded = {}
    for k, v in inputs.items():
        a = np.ones((BPAD, 3), dtype=np.float32)
        a[:B] = v
        padded[k] = a
    args = [padded[n] for n in in_names]
    zeros = [np.zeros((N_CORES * z.shape[0], *z.shape[1:]), z.dtype) for z in zero_outs]
    outs = jf(*args, *zeros)
    full = np.asarray(outs[0])  # [BPAD, 12]
    return full[:B].reshape(B, 3, 4).astype(np.float32, copy=False)


if __name__ == "__main__":
    rng = np.random.default_rng(0)
    ins = {
        "trans": rng.normal(size=(B, 3)).astype(np.float32),
        "rotat": rng.normal(size=(B, 3)).astype(np.float32),
        "scal_dir": rng.normal(size=(B, 3)).astype(np.float32),
        "scal": rng.normal(size=(B, 3)).astype(np.float32),
    }
    out = kernel(**ins)
    print(out.shape, out.dtype)
